# revision 8
# baseline (speedup 1.0000x reference)
"""Trainium2 Bass kernel for nn_MultiHeadAttention (B=2, S=2048, DM=1024, H=8).

Sharding: data-parallel on batch x tensor-parallel on heads.
Core c in 0..7 handles batch b = c//4 and heads {2*(c%4), 2*(c%4)+1}.
Each core computes its two heads' full attention and the partial
out-projection (a 1024x2048 partial sum in fp16); the host adds the 4
partials per batch and transposes back to (S, DOUT).

v2 restructure vs the original baseline (which stalled the PE ~80us):
  - x ring of 16 bufs (was 9): t0/t1 x tiles stream ungated; t2 (V) x
    DMAs release early enough to avoid the 16us V-projection stall.
  - mask shipped as fp8e4 {0,1}: halves mask DMA (8.4->4.2MB) and SBUF.
  - input DMAs only on sync/gpsimd/vector queues; scalar carries no DMA
    so ACT evacuations can never deadlock behind gated x DMAs. Program
    order per queue: x/w first, then mask, then wo (priority by need).
  - PSUM evacuation with bias moved from DVE tensor_scalar to ACT
    Identity (bias=per-partition AP): balances Vector/Scalar load.
  - reciprocal chain: DVE reciprocal directly on PSUM rowsum rows 0/32,
    one SBUF->SBUF broadcast DMA (was 3 DMA hops through DRAM).
  - out-projection interleaved per half: its PE work fills the
    half-boundary gap and output DMA starts at ~60% of the kernel.
  - outT in fp16 (halves output DMA; host sums partials in fp32).
"""

import sys

sys.path.insert(0, "/opt/trn_rl_repo")

import numpy as np
import ml_dtypes

import concourse.bass as bass
import concourse.tile as tile
from concourse import bacc, mybir
from concourse.bass import ts, ds
from concourse.bass_utils import run_bass_kernel_spmd

BF16 = mybir.dt.bfloat16
F32 = mybir.dt.float32
F16 = mybir.dt.float16
FP8 = mybir.dt.float8e4
Exp = mybir.ActivationFunctionType.Exp
Ident = mybir.ActivationFunctionType.Identity

B, S, DM, H, DOUT = 2, 2048, 1024, 8, 1024
D = DM // H            # 128 head dim
NH = 2                 # heads per core
KC = DM // 128         # 8 contraction chunks for projections
OC = S // 128          # 16 key chunks
NT = 512               # PSUM-bank-sized free tile (fp32)
IT = S // NT           # 4 query tiles
SCALE = float(1.0 / np.sqrt(np.float32(D)))


def build():
    nc = bacc.Bacc(None, target_bir_lowering=False)

    xT = nc.dram_tensor("xT", [3, DM, S], BF16, kind="ExternalInput")
    maskT8 = nc.dram_tensor("maskT8", [S, S], FP8, kind="ExternalInput")
    w_qkv = nc.dram_tensor("w_qkv", [128, 3, KC, NH, D], BF16, kind="ExternalInput")
    b_qkv = nc.dram_tensor("b_qkv", [128, 3, NH], F32, kind="ExternalInput")
    wo = nc.dram_tensor("wo", [D, NH, DOUT], BF16, kind="ExternalInput")
    ident = nc.dram_tensor("ident", [128, 128], BF16, kind="ExternalInput")
    bo = nc.dram_tensor("bo", [128, DOUT // 128], F32, kind="ExternalInput")
    outT = nc.dram_tensor("outT", [DOUT, S], F16, kind="ExternalOutput")

    # input DMA queues: scalar may only carry UNGATED preamble DMAs (its
    # ACT evacuations must never sit behind a ring-gated DMA's semaphore
    # wait); gated t2-x and mask transfers go on sync/gpsimd only.
    q_sg = [nc.sync, nc.gpsimd]
    q_sgs = [nc.sync, nc.gpsimd, nc.scalar]

    with tile.TileContext(nc) as tc:
        with (
            tc.tile_pool(name="const", bufs=1) as constp,
            tc.tile_pool(name="xin", bufs=16) as xp,
            tc.tile_pool(name="ptile", bufs=4) as pp,
            tc.tile_pool(name="rb", bufs=2) as rbp,
            tc.tile_pool(name="fout", bufs=4) as fop,
            tc.tile_pool(name="psum", bufs=4, space="PSUM") as psp,
            tc.tile_pool(name="dram", bufs=2, space="DRAM") as dramp,
        ):
            # ---- small constants (scalar queue; ungated, tiny) ----
            b_sb = constp.tile([128, 3, NH], F32)
            nc.scalar.dma_start(out=b_sb, in_=b_qkv[:])
            bo_sb = constp.tile([128, DOUT // 128], F32)
            nc.scalar.dma_start(out=bo_sb, in_=bo[:])
            ident_sb = constp.tile([128, 128], BF16)
            nc.scalar.dma_start(out=ident_sb, in_=ident[:])
            ones_col = constp.tile([128, 1], BF16)
            nc.vector.memset(ones_col, 1.0)

            # ---- weights + x tiles interleaved in need order ----
            w_sb = constp.tile([128, 3, KC, NH, D], BF16)
            wo_sb = constp.tile([D, NH, DOUT], BF16)
            xts_all = []
            for t in range(3):
                row = []
                for k in range(KC):
                    qi = t * KC + k
                    if t < 2:
                        q_w = q_sgs[(qi + 1) % 3]
                        q_x = q_sgs[qi % 3]
                    else:
                        # t2 x DMAs are ring-gated: sync/gpsimd only; their
                        # (ungated) weights go early on scalar.
                        q_w = nc.scalar
                        q_x = q_sg[qi % 2]
                    q_w.dma_start(out=w_sb[:, t, k, :, :], in_=w_qkv[:, t, k, :, :])
                    xt = xp.tile([128, S], BF16, tag="x", name=f"x{t}_{k}")
                    q_x.dma_start(out=xt, in_=xT[t, k * 128 : (k + 1) * 128, :])
                    row.append(xt)
                xts_all.append(row)

            # mask after all x in program order: in-order queues give x
            # priority; mask lands during the projection phase.
            mask_sb = constp.tile([128, OC, S], FP8)
            for oc in range(OC):
                q_sg[oc % 2].dma_start(
                    out=mask_sb[:, oc, :], in_=maskT8[oc * 128 : (oc + 1) * 128, :]
                )
            nc.scalar.dma_start(out=wo_sb, in_=wo[:])

            # ---- Q/K projections: qk_sb[d, t, h, s]; bias added on ACT evac ----
            qk_sb = constp.tile([128, 2, NH, S], BF16)
            for t in range(2):
                xts = xts_all[t]
                for h in range(NH):
                    acc = [
                        psp.tile([128, NT], F32, tag="acc", name=f"acc{it}")
                        for it in range(IT)
                    ]
                    for k in range(KC):
                        for it in range(IT):
                            nc.tensor.matmul(
                                acc[it],
                                w_sb[:, t, k, h, :],
                                xts[k][:, ts(it, NT)],
                                start=(k == 0),
                                stop=(k == KC - 1),
                            )
                    for it in range(IT):
                        nc.scalar.activation(
                            out=qk_sb[:, t, h, ts(it, NT)],
                            in_=acc[it],
                            func=Ident,
                            bias=b_sb[:, t, h : h + 1],
                        )

            # ---- V projection via VpT + PE transpose: vp_sb[s%128, oc, h, d] ----
            vpt_sb = constp.tile([128, NH, S], BF16)  # [d, h, s] transient
            vp_sb = constp.tile([128, OC, NH, D], BF16)
            xts = xts_all[2]
            for h in range(NH):
                acc = [
                    psp.tile([128, NT], F32, tag="acc", name=f"vacc{it}")
                    for it in range(IT)
                ]
                for k in range(KC):
                    for it in range(IT):
                        nc.tensor.matmul(
                            acc[it],
                            w_sb[:, 2, k, h, :],
                            xts[k][:, ts(it, NT)],
                            start=(k == 0),
                            stop=(k == KC - 1),
                        )
                for it in range(IT):
                    nc.scalar.activation(
                        out=vpt_sb[:, h, ts(it, NT)],
                        in_=acc[it],
                        func=Ident,
                        bias=b_sb[:, 2, h : h + 1],
                    )
                for oc in range(OC):
                    tps = psp.tile([128, D], BF16, tag="s", bufs=2, name="tps")
                    nc.tensor.transpose(
                        tps, vpt_sb[:, h, ds(oc * 128, 128)], ident_sb
                    )
                    nc.vector.tensor_copy(vp_sb[:, oc, h, :], tps)

            # ---- attention per half (1024 queries) / per head,
            #      out-projection interleaved after each half ----
            outn_sb = constp.tile([128, NH, S], BF16)
            for half in range(2):
                for h in range(NH):
                    i0 = half * 2 * NT
                    outp = [
                        psp.tile([128, NT], F32, tag="acc", name=f"outp{j}")
                        for j in range(2)
                    ]
                    rp = psp.tile([128, NT], F32, tag="acc", name="rp")
                    for oc in range(OC):
                        sps = psp.tile(
                            [128, 2 * NT], F32, tag="s", bufs=2, name="sps"
                        )
                        for j in range(2):
                            nc.tensor.matmul(
                                sps[:, ts(j, NT)],
                                qk_sb[:, 1, h, ds(oc * 128, 128)],
                                qk_sb[:, 0, h, ds(i0 + j * NT, NT)],
                                start=True,
                                stop=True,
                            )
                        p = pp.tile([128, 2 * NT], BF16, tag="p")
                        nc.scalar.activation(
                            out=p, in_=sps, func=Exp, bias=0.0, scale=SCALE
                        )
                        pm = pp.tile([128, 2 * NT], BF16, tag="pm")
                        nc.vector.tensor_mul(
                            pm, p, mask_sb[:, oc, ds(i0, 2 * NT)]
                        )
                        for j in range(2):
                            nc.tensor.matmul(
                                outp[j],
                                vp_sb[:, oc, h, :],
                                pm[:, ts(j, NT)],
                                start=(oc == 0),
                                stop=(oc == OC - 1),
                            )
                        for j in range(2):
                            nc.tensor.matmul(
                                rp[32 * j : 32 * j + 1, :],
                                ones_col,
                                pm[:, ts(j, NT)],
                                start=(oc == 0),
                                stop=(oc == OC - 1),
                            )
                    # evacuate PSUM: out tiles to SBUF, reciprocal straight
                    # off the rowsum rows, single SBUF->SBUF broadcast DMA
                    osb = rbp.tile([128, 2 * NT], F32, tag="osb")
                    for j in range(2):
                        nc.vector.tensor_copy(osb[:, ts(j, NT)], outp[j])
                    rec2 = rbp.tile([33, NT], F32, tag="rec2")
                    rd = dramp.tile([2, NT], F32, tag="rd")
                    for j in range(2):
                        nc.vector.reciprocal(
                            rec2[32 * j : 32 * j + 1, :],
                            rp[32 * j : 32 * j + 1, :],
                        )
                        q_sg[j].dma_start(
                            out=rd[j : j + 1, :],
                            in_=rec2[32 * j : 32 * j + 1, :],
                        )
                    for j in range(2):
                        rbc = rbp.tile([128, NT], F32, tag="rbc", bufs=2)
                        q_sg[j].dma_start(
                            out=rbc,
                            in_=rd[j : j + 1, :].to_broadcast([128, NT]),
                        )
                        nc.vector.tensor_mul(
                            outn_sb[:, h, ds(i0 + j * NT, NT)],
                            osb[:, ts(j, NT)],
                            rbc,
                        )
                # out-projection for this half's two query tiles
                for it in (2 * half, 2 * half + 1):
                    for dc in range(DOUT // 128):
                        facc = psp.tile([128, NT], F32, tag="acc")
                        for h in range(NH):
                            nc.tensor.matmul(
                                facc,
                                wo_sb[:, h, ds(dc * 128, 128)],
                                outn_sb[:, h, ts(it, NT)],
                                start=(h == 0),
                                stop=(h == NH - 1),
                            )
                        fsb = fop.tile([128, NT], F16, tag="f")
                        nc.vector.tensor_scalar_add(
                            out=fsb, in0=facc, scalar1=bo_sb[:, dc : dc + 1]
                        )
                        q_sg[dc % 2].dma_start(
                            out=outT[dc * 128 : (dc + 1) * 128, ts(it, NT)],
                            in_=fsb,
                        )

    return nc


_NC_CACHE = None


def _get_nc():
    global _NC_CACHE
    if _NC_CACHE is None:
        nc = build()
        nc.compile()
        _NC_CACHE = nc
    return _NC_CACHE


def make_in_maps(q, k, v, mask, Wq, bq, Wk, bk, Wv, bv, Wo, bo):
    bf = ml_dtypes.bfloat16
    f8 = ml_dtypes.float8_e4m3
    q = np.asarray(q, np.float32)
    k = np.asarray(k, np.float32)
    v = np.asarray(v, np.float32)
    mask = np.asarray(mask)
    Ws = [np.asarray(w, np.float32) for w in (Wq, Wk, Wv)]
    bs = [np.asarray(b_, np.float32) for b_ in (bq, bk, bv)]
    Wo = np.asarray(Wo, np.float32)
    bo = np.asarray(bo, np.float32)

    xTb, maskTb = [], []
    for b in range(B):
        xTb.append(
            np.ascontiguousarray(np.stack([q[b].T, k[b].T, v[b].T]).astype(bf))
        )
        maskTb.append(
            np.ascontiguousarray(mask[b].T.astype(np.float32)).astype(f8)
        )
    # W[dm, dout] with head h owning columns d*H+h; reshape for tile slicing:
    # Wr[t][kc, p, d, h] = W[kc*128+p, d*H+h]
    Wr = [W.reshape(KC, 128, D, H) for W in Ws]
    br = [b_.reshape(D, H) for b_ in bs]

    ident = np.eye(128, dtype=np.float32).astype(bf)

    in_maps = []
    for c in range(8):
        b = c // 4
        h0 = NH * (c % 4)
        w_core = np.empty((128, 3, KC, NH, D), np.float32)
        for t in range(3):
            for hi in range(NH):
                w_core[:, t, :, hi, :] = Wr[t][:, :, :, h0 + hi].transpose(1, 0, 2)
        # per-partition (d) bias columns for the ACT evacuations
        b_core = np.empty((128, 3, NH), np.float32)
        for t in range(3):
            for hi in range(NH):
                b_core[:, t, hi] = br[t][:, h0 + hi]
        wo_core = np.stack([Wo[h0 + hi :: H, :] for hi in range(NH)], axis=1)
        bo_core = bo if c % 4 == 0 else np.zeros_like(bo)
        in_maps.append(
            {
                "xT": xTb[b],
                "ident": ident,
                "maskT8": maskTb[b],
                "w_qkv": np.ascontiguousarray(w_core).astype(bf),
                "b_qkv": np.ascontiguousarray(b_core),
                "wo": np.ascontiguousarray(wo_core).astype(bf),
                "bo": np.ascontiguousarray(bo_core.reshape(DOUT // 128, 128).T),
            }
        )
    return in_maps


def unshard(results):
    out = np.zeros((B, DOUT, S), np.float32)
    for c in range(8):
        out[c // 4] += np.asarray(results[c]["outT"], np.float32)
    return np.ascontiguousarray(out.transpose(0, 2, 1))


def kernel(**inputs):
    in_maps = make_in_maps(**inputs)
    nc = _get_nc()
    res = run_bass_kernel_spmd(nc, in_maps, core_ids=list(range(8)))
    return unshard(res.results)


# revision 16
# speedup vs baseline: 1.2178x; 1.2178x over previous
"""Trainium2 Bass kernel for nn_MultiHeadAttention (B=2, S=2048, DM=1024, H=8).

Sharding: data-parallel on batch x tensor-parallel on heads.
Core c in 0..7 handles batch b = c//4 and heads {2*(c%4), 2*(c%4)+1}.
Each core computes its two heads' full attention and the partial
out-projection (a 1024x2048 partial sum in fp16); the host adds the 4
partials per batch and transposes back to (S, DOUT).

v3 structure (why):
  - the kernel is PE-streaming-bound (~140us of bf16 matmul columns);
    everything else is scheduled to keep the PE queue from ever waiting.
  - x ring of 14 bufs; w/x interleaved in need order on 3 DMA queues.
  - mask is bf16 (DVE mask-multiply stays in 2x mode; fp8 operands
    halve DVE throughput) but shipped as 32 half-column tiles
    [128,1024] through a 20-buf ring: only half0's 4.2MB must land
    before attention starts; half1's tiles stream during half0's
    attention when HBM is otherwise idle.
  - scalar queue carries only ungated DMAs (its ACT evacuations can
    never sit behind a ring-gated DMA semaphore); gated x-t2 and
    half1-mask DMAs live on sync/gpsimd.
  - PSUM bias-evacuation on ACT (Identity + per-partition bias AP).
  - reciprocal on a [128,8]-repacked layout (DVE reciprocal cost is
    its free-dim size: [1,512] would be ~4us, [128,8] is ~0.3us).
  - out-projection of half0 is software-pipelined one tile per oc
    into half1-h0's attention loop so its PE/DVE work hides under the
    attention stream instead of serializing after it; half1's
    out-projection forms the tail with alternating DVE/ACT evacs.
  - outT in fp16 (halves output DMA; host sums partials in fp32).
PSUM budget (8 banks): tag acc 3 + tag s 2x[128,1024] = 4 + facc 1.
Projections therefore accumulate 2 query-tiles at a time (2 groups).
"""

import sys

sys.path.insert(0, "/opt/trn_rl_repo")

import numpy as np
import ml_dtypes

import concourse.bass as bass
import concourse.tile as tile
from concourse import bacc, mybir
from concourse.bass import ts, ds
from concourse.bass_utils import run_bass_kernel_spmd

BF16 = mybir.dt.bfloat16
F32 = mybir.dt.float32
F16 = mybir.dt.float16
Exp = mybir.ActivationFunctionType.Exp
Ident = mybir.ActivationFunctionType.Identity

B, S, DM, H, DOUT = 2, 2048, 1024, 8, 1024
D = DM // H            # 128 head dim
NH = 2                 # heads per core
KC = DM // 128         # 8 contraction chunks for projections
OC = S // 128          # 16 key chunks
NT = 512               # PSUM-bank-sized free tile (fp32)
IT = S // NT           # 4 query tiles
SCALE = float(1.0 / np.sqrt(np.float32(D)))


def build():
    nc = bacc.Bacc(None, target_bir_lowering=False)

    xT = nc.dram_tensor("xT", [3, DM, S], BF16, kind="ExternalInput")
    maskT = nc.dram_tensor("maskT", [S, S], BF16, kind="ExternalInput")
    w_qkv = nc.dram_tensor("w_qkv", [128, 3, KC, NH, D], BF16, kind="ExternalInput")
    b_qkv = nc.dram_tensor("b_qkv", [128, 3, NH], F32, kind="ExternalInput")
    wo = nc.dram_tensor("wo", [D, NH, DOUT], BF16, kind="ExternalInput")
    ident = nc.dram_tensor("ident", [128, 128], BF16, kind="ExternalInput")
    bo = nc.dram_tensor("bo", [128, DOUT // 128], F32, kind="ExternalInput")
    outT = nc.dram_tensor("outT", [DOUT, S], F16, kind="ExternalOutput")

    q_sg = [nc.sync, nc.gpsimd]
    q_sgs = [nc.sync, nc.gpsimd, nc.scalar]

    with tile.TileContext(nc) as tc:
        with (
            tc.tile_pool(name="const", bufs=1) as constp,
            tc.tile_pool(name="xin", bufs=14) as xp,
            tc.tile_pool(name="mask", bufs=20) as mp,
            tc.tile_pool(name="ptile", bufs=4) as pp,
            tc.tile_pool(name="rb", bufs=2) as rbp,
            tc.tile_pool(name="fout", bufs=4) as fop,
            tc.tile_pool(name="psum", bufs=3, space="PSUM") as psp,
            tc.tile_pool(name="dram", bufs=2, space="DRAM") as dramp,
        ):
            # ---- small constants (scalar queue; ungated, tiny) ----
            b_sb = constp.tile([128, 3, NH], F32)
            nc.scalar.dma_start(out=b_sb, in_=b_qkv[:])
            bo_sb = constp.tile([128, DOUT // 128], F32)
            nc.scalar.dma_start(out=bo_sb, in_=bo[:])
            ident_sb = constp.tile([128, 128], BF16)
            nc.scalar.dma_start(out=ident_sb, in_=ident[:])
            ones_col = constp.tile([128, 1], BF16)
            nc.vector.memset(ones_col, 1.0)

            # ---- weights + x tiles interleaved in need order ----
            w_sb = constp.tile([128, 3, KC, NH, D], BF16)
            wo_sb = constp.tile([D, NH, DOUT], BF16)
            xts_all = []
            for t in range(3):
                row = []
                for k in range(KC):
                    qi = t * KC + k
                    if t < 2:
                        q_w = q_sgs[(qi + 1) % 3]
                        q_x = q_sgs[qi % 3]
                    else:
                        # ring-gated x-t2: sync/gpsimd only; its (ungated)
                        # weights go early on scalar.
                        q_w = nc.scalar
                        q_x = q_sg[qi % 2]
                    q_w.dma_start(out=w_sb[:, t, k, :, :], in_=w_qkv[:, t, k, :, :])
                    xt = xp.tile([128, S], BF16, tag="x", name=f"x{t}_{k}")
                    q_x.dma_start(out=xt, in_=xT[t, k * 128 : (k + 1) * 128, :])
                    row.append(xt)
                xts_all.append(row)

            # mask half-column tiles [128,1024] keyed (half, oc). half0's 16
            # are queued here behind x-t2 (sync/gpsimd, few on scalar);
            # half1's are allocated+issued lazily inside half0-h1's
            # attention loop so they enter the in-order queues after the
            # half0 reciprocal-chain DMAs and stream while HBM is idle.
            mask_t = {}

            def mask_fetch(half, oc, qm):
                mt = mp.tile([128, 2 * NT], BF16, tag="m", name=f"m{half}_{oc}")
                qm.dma_start(
                    out=mt,
                    in_=maskT[
                        oc * 128 : (oc + 1) * 128,
                        half * 2 * NT : (half + 1) * 2 * NT,
                    ],
                )
                mask_t[(half, oc)] = mt

            for oc in range(OC):
                qm = nc.scalar if oc % 4 == 3 else q_sg[oc % 2]
                mask_fetch(0, oc, qm)
            nc.scalar.dma_start(out=wo_sb, in_=wo[:])

            # ---- Q/K projections: qk_sb[d, t, h, s] (2 query-tile groups,
            #      3-buf PSUM acc ring); bias added on ACT evac ----
            qk_sb = constp.tile([128, 2, NH, S], BF16)
            for t in range(2):
                xts = xts_all[t]
                for h in range(NH):
                    for g in range(2):
                        acc = [
                            psp.tile([128, NT], F32, tag="acc", name=f"acc{g}{i}")
                            for i in range(2)
                        ]
                        for k in range(KC):
                            for i in range(2):
                                nc.tensor.matmul(
                                    acc[i],
                                    w_sb[:, t, k, h, :],
                                    xts[k][:, ts(2 * g + i, NT)],
                                    start=(k == 0),
                                    stop=(k == KC - 1),
                                )
                        for i in range(2):
                            nc.scalar.activation(
                                out=qk_sb[:, t, h, ts(2 * g + i, NT)],
                                in_=acc[i],
                                func=Ident,
                                bias=b_sb[:, t, h : h + 1],
                            )

            # ---- V projection via VpT + PE transpose: vp_sb[s%128, oc, h, d] ----
            vpt_sb = constp.tile([128, NH, S], BF16)  # [d, h, s] transient
            vp_sb = constp.tile([128, OC, NH, D], BF16)
            xts = xts_all[2]
            for h in range(NH):
                for g in range(2):
                    acc = [
                        psp.tile([128, NT], F32, tag="acc", name=f"vacc{g}{i}")
                        for i in range(2)
                    ]
                    for k in range(KC):
                        for i in range(2):
                            nc.tensor.matmul(
                                acc[i],
                                w_sb[:, 2, k, h, :],
                                xts[k][:, ts(2 * g + i, NT)],
                                start=(k == 0),
                                stop=(k == KC - 1),
                            )
                    for i in range(2):
                        nc.scalar.activation(
                            out=vpt_sb[:, h, ts(2 * g + i, NT)],
                            in_=acc[i],
                            func=Ident,
                            bias=b_sb[:, 2, h : h + 1],
                        )
                for oc in range(OC):
                    tps = psp.tile([128, D], BF16, tag="s", bufs=2, name="tps")
                    nc.tensor.transpose(
                        tps, vpt_sb[:, h, ds(oc * 128, 128)], ident_sb
                    )
                    nc.vector.tensor_copy(vp_sb[:, oc, h, :], tps)

            outn_sb = constp.tile([128, NH, S], BF16)

            def outproj_item(it, dc, facc_tag, on_act, facc_bufs=1):
                """One out-projection tile: facc = sum_h wo_h^T outn_h."""
                facc = psp.tile(
                    [128, NT], F32, tag=facc_tag, bufs=facc_bufs, name="facc"
                )
                for h in range(NH):
                    nc.tensor.matmul(
                        facc,
                        wo_sb[:, h, ds(dc * 128, 128)],
                        outn_sb[:, h, ts(it, NT)],
                        start=(h == 0),
                        stop=(h == NH - 1),
                    )
                fsb = fop.tile([128, NT], F16, tag="f")
                if on_act:
                    nc.scalar.activation(
                        out=fsb, in_=facc, func=Ident,
                        bias=bo_sb[:, dc : dc + 1],
                    )
                else:
                    nc.vector.tensor_scalar_add(
                        out=fsb, in0=facc, scalar1=bo_sb[:, dc : dc + 1]
                    )
                q_sg[dc % 2].dma_start(
                    out=outT[dc * 128 : (dc + 1) * 128, ts(it, NT)], in_=fsb
                )

            def attention(half, h, inject, prefetch=False):
                """One head's attention over 1024 queries; `inject` is a list
                of deferred out-projection items, one emitted per oc;
                `prefetch` issues the next half's mask DMAs one per oc."""
                i0 = half * 2 * NT
                outp = [
                    psp.tile([128, NT], F32, tag="acc", name=f"outp{j}")
                    for j in range(2)
                ]
                rp = psp.tile([128, NT], F32, tag="acc", name="rp")
                for oc in range(OC):
                    sps = psp.tile([128, 2 * NT], F32, tag="s", bufs=2, name="sps")
                    for j in range(2):
                        nc.tensor.matmul(
                            sps[:, ts(j, NT)],
                            qk_sb[:, 1, h, ds(oc * 128, 128)],
                            qk_sb[:, 0, h, ds(i0 + j * NT, NT)],
                            start=True,
                            stop=True,
                        )
                    p = pp.tile([128, 2 * NT], BF16, tag="p")
                    nc.scalar.activation(
                        out=p, in_=sps, func=Exp, bias=0.0, scale=SCALE
                    )
                    pm = pp.tile([128, 2 * NT], BF16, tag="pm")
                    nc.vector.tensor_mul(pm, p, mask_t[(half, oc)])
                    for j in range(2):
                        nc.tensor.matmul(
                            outp[j],
                            vp_sb[:, oc, h, :],
                            pm[:, ts(j, NT)],
                            start=(oc == 0),
                            stop=(oc == OC - 1),
                        )
                    for j in range(2):
                        nc.tensor.matmul(
                            rp[32 * j : 32 * j + 1, :],
                            ones_col,
                            pm[:, ts(j, NT)],
                            start=(oc == 0),
                            stop=(oc == OC - 1),
                        )
                    if prefetch:
                        mask_fetch(half + 1, oc, q_sg[oc % 2])
                    if inject:
                        outproj_item(*inject.pop(0), "facc", False, 1)
                # PSUM evacuation + softmax normalization. The reciprocal runs
                # on a [128,8] repack (DVE reciprocal cost = free-dim size).
                osb = rbp.tile([128, 2 * NT], F32, tag="osb")
                for j in range(2):
                    nc.vector.tensor_copy(osb[:, ts(j, NT)], outp[j])
                r2 = rbp.tile([33, NT], F32, tag="r2")
                for j in range(2):
                    nc.vector.tensor_copy(
                        r2[32 * j : 32 * j + 1, :], rp[32 * j : 32 * j + 1, :]
                    )
                rd = dramp.tile([2, NT], F32, tag="rd")
                for j in range(2):
                    q_sg[j].dma_start(
                        out=rd[j : j + 1, :], in_=r2[32 * j : 32 * j + 1, :]
                    )
                rseg = rbp.tile([128, 8], F32, tag="rseg")
                nc.sync.dma_start(
                    out=rseg,
                    in_=rd[:].rearrange("a b -> (a b)").rearrange(
                        "(p j) -> p j", p=128
                    ),
                )
                nc.vector.reciprocal(rseg, rseg)
                rd2 = dramp.tile([2, NT], F32, tag="rd2")
                nc.gpsimd.dma_start(
                    out=rd2[:].rearrange("a b -> (a b)").rearrange(
                        "(p j) -> p j", p=128
                    ),
                    in_=rseg,
                )
                for j in range(2):
                    rbc = rbp.tile([128, NT], F32, tag="rbc", bufs=2)
                    q_sg[j].dma_start(
                        out=rbc, in_=rd2[j : j + 1, :].to_broadcast([128, NT])
                    )
                    nc.vector.tensor_mul(
                        outn_sb[:, h, ds(i0 + j * NT, NT)],
                        osb[:, ts(j, NT)],
                        rbc,
                    )

            # half0 attention; its out-projection rides inside half1-h0.
            attention(0, 0, [])
            attention(0, 1, [], prefetch=True)
            half0_items = [(it, dc) for it in (0, 1) for dc in range(DOUT // 128)]
            attention(1, 0, half0_items)
            attention(1, 1, [])
            # tail: half1 out-projection, alternating DVE/ACT evacuation,
            # 3 PSUM tiles in flight (facc + the two freed s-tag slots).
            for n, (it, dc) in enumerate(
                (it, dc) for it in (2, 3) for dc in range(DOUT // 128)
            ):
                if n % 3 == 0:
                    outproj_item(it, dc, "facc", n % 2 == 1, 1)
                else:
                    outproj_item(it, dc, "s", n % 2 == 1, 2)

    return nc


_NC_CACHE = None


def _get_nc():
    global _NC_CACHE
    if _NC_CACHE is None:
        nc = build()
        nc.compile()
        _NC_CACHE = nc
    return _NC_CACHE


def make_in_maps(q, k, v, mask, Wq, bq, Wk, bk, Wv, bv, Wo, bo):
    bf = ml_dtypes.bfloat16
    q = np.asarray(q, np.float32)
    k = np.asarray(k, np.float32)
    v = np.asarray(v, np.float32)
    mask = np.asarray(mask)
    Ws = [np.asarray(w, np.float32) for w in (Wq, Wk, Wv)]
    bs = [np.asarray(b_, np.float32) for b_ in (bq, bk, bv)]
    Wo = np.asarray(Wo, np.float32)
    bo = np.asarray(bo, np.float32)

    xTb, maskTb = [], []
    for b in range(B):
        xTb.append(
            np.ascontiguousarray(np.stack([q[b].T, k[b].T, v[b].T]).astype(bf))
        )
        maskTb.append(
            np.ascontiguousarray(mask[b].T.astype(np.float32)).astype(bf)
        )
    # W[dm, dout] with head h owning columns d*H+h; reshape for tile slicing:
    # Wr[t][kc, p, d, h] = W[kc*128+p, d*H+h]
    Wr = [W.reshape(KC, 128, D, H) for W in Ws]
    br = [b_.reshape(D, H) for b_ in bs]

    ident = np.eye(128, dtype=np.float32).astype(bf)

    in_maps = []
    for c in range(8):
        b = c // 4
        h0 = NH * (c % 4)
        w_core = np.empty((128, 3, KC, NH, D), np.float32)
        for t in range(3):
            for hi in range(NH):
                w_core[:, t, :, hi, :] = Wr[t][:, :, :, h0 + hi].transpose(1, 0, 2)
        # per-partition (d) bias columns for the ACT evacuations
        b_core = np.empty((128, 3, NH), np.float32)
        for t in range(3):
            for hi in range(NH):
                b_core[:, t, hi] = br[t][:, h0 + hi]
        wo_core = np.stack([Wo[h0 + hi :: H, :] for hi in range(NH)], axis=1)
        bo_core = bo if c % 4 == 0 else np.zeros_like(bo)
        in_maps.append(
            {
                "xT": xTb[b],
                "ident": ident,
                "maskT": maskTb[b],
                "w_qkv": np.ascontiguousarray(w_core).astype(bf),
                "b_qkv": np.ascontiguousarray(b_core),
                "wo": np.ascontiguousarray(wo_core).astype(bf),
                "bo": np.ascontiguousarray(bo_core.reshape(DOUT // 128, 128).T),
            }
        )
    return in_maps


def unshard(results):
    out = np.zeros((B, DOUT, S), np.float32)
    for c in range(8):
        out[c // 4] += np.asarray(results[c]["outT"], np.float32)
    return np.ascontiguousarray(out.transpose(0, 2, 1))


def kernel(**inputs):
    in_maps = make_in_maps(**inputs)
    nc = _get_nc()
    res = run_bass_kernel_spmd(nc, in_maps, core_ids=list(range(8)))
    return unshard(res.results)
